# revision 5
# baseline (speedup 1.0000x reference)
"""Channel-attention module kernel for 8 Trainium2 NeuronCores.

reference semantics (B=2, C=128, N=D*H*W=147456):
    q = x.reshape(B, C, N)
    energy = q @ q^T                  # [B, C, C]
    attn = softmax(rowmax(energy) - energy, axis=-1)
          = softmax(-energy, axis=-1)             (rowmax shift is a no-op)
    out = attn @ q
    return x + gamma * out

Sharding: sequence-parallel over N. Core r owns columns
[r*N/8, (r+1)*N/8) of q for both batches. Each core computes a partial
energy (contraction over its local n), ONE AllReduce sums the two tiny
[C, C] energies (packed as [C, 2C]) across the 8 cores, each core then
computes the softmax redundantly and applies the attention to its
local columns.

Design notes (v3):
  - Host pre-splits q into f16 hi/lo (q = hi + lo + O(2^-22)) shipped
    PRE-TRANSPOSED in an SBUF-image blocked layout. Phase 1 is pure
    f16 PE work (1 cyc/row): E1 += hi_t^T hi_t, M += hi_t^T lo_t per
    128-tile; cross term completed once per batch as E = E1 + M + M^T.
    End-to-end rel err 3.3e-4 (numpy-validated; gate 2e-2).
  - [c, n] f16 q copy for phase 2 rebuilt on-chip via f16 PE
    transposes; PSUM->SBUF copies split ScalarE/DVE.
  - P-state: TRN2 PE runs 1.2GHz until ~3us of CONTINUOUS busy, then
    2.4GHz. A DMA-paced phase 1 keeps stalling PE and pins it at
    1.2GHz (measured). So PE start is delayed (dummy transpose gated
    on chunk DELAY's arrival) until enough input is buffered that PE
    then runs gap-free at 2.4GHz to the end of phase 1.
  - hi loads on the sync HWDGE queue, lo loads on the scalar queue,
    output stores alternate sync/scalar: two queues split the HBM
    traffic. Small tensors (identities, gamma) load via the scalar
    queue so they don't head-of-line block the bulk stream.
  - ONE AllReduce [C, 2C] fp32; warm-up collective fired dep-free at
    t=0 (the first collective pays a ~60-85us ncfw cold-start
    pipeline; hw-measured).
  - softmax critical path is just min -> exp(accum Z) -> transpose:
    the gamma/Z row scale and the +q residual are fused into the
    phase-2 PSUM->SBUF copies as (op * scol) + q_f16 on DVE.
"""

import sys

sys.path.insert(0, "/opt/trn_rl_repo")

import numpy as np

B, C = 2, 128
D, H, W = 16, 96, 96
N = D * H * W  # 147456
NCORES = 8
NLOC = N // NCORES  # 18432
CHUNK = 2048
NCHUNK_B = NLOC // CHUNK  # 9 chunks per batch
NCHUNK = B * NCHUNK_B  # 18
TPC = CHUNK // C  # 16 n-tiles of 128 per chunk
OTILE = 512
DELAY = 4  # chunks buffered before PE starts (p-state ramp guard)

_compiled = {}


def _log(msg):
    import time as _t
    print(f"[kernel {_t.strftime('%H:%M:%S')}] {msg}", flush=True)


def _build():
    import concourse.bacc as bacc
    import concourse.tile as tile
    import concourse.mybir as mybir

    _log("build start")

    f32 = mybir.dt.float32
    f16 = mybir.dt.float16
    nc = bacc.Bacc("TRN2", target_bir_lowering=False, debug=False,
                   num_devices=NCORES)

    hi_d = nc.dram_tensor("hi", [NCHUNK, C, CHUNK], f16,
                          kind="ExternalInput").ap()
    lo_d = nc.dram_tensor("lo", [NCHUNK, C, CHUNK], f16,
                          kind="ExternalInput").ap()
    g_d = nc.dram_tensor("gamma_col", [C, 1], f32, kind="ExternalInput").ap()
    id_d = nc.dram_tensor("ident", [C, C], f32, kind="ExternalInput").ap()
    idh_d = nc.dram_tensor("identh", [C, C], f16, kind="ExternalInput").ap()
    o_d = nc.dram_tensor("out", [B, C, NLOC], f16, kind="ExternalOutput").ap()

    with tile.TileContext(nc) as tc:
        with (
            tc.tile_pool(name="hip", bufs=8) as hip,
            tc.tile_pool(name="lop", bufs=8) as lop,
            tc.tile_pool(name="xb16", bufs=NCHUNK) as xbp,
            tc.tile_pool(name="tps", bufs=2, space="PSUM") as tps,
            tc.tile_pool(name="tp2", bufs=1, space="PSUM") as tp2p,
            tc.tile_pool(name="eps", bufs=2, space="PSUM") as eps,
            tc.tile_pool(name="ops", bufs=3, space="PSUM") as ops,
            tc.tile_pool(name="misc", bufs=1) as mp,
            tc.tile_pool(name="ost", bufs=4) as ostp,
            tc.tile_pool(name="dram", bufs=1, space="DRAM") as dramp,
        ):
            # ---- warm-up collective, dependency-free, at t=0 ----
            wz = mp.tile([C, 1], f32, name="wz")
            nc.gpsimd.memzero(wz[:])
            w_in = dramp.tile([C, 1], f32, name="w_in")
            w_out = dramp.tile([C, 1], f32, name="w_out", addr_space="Shared")
            nc.gpsimd.dma_start(w_in[:], wz[:])
            nc.gpsimd.collective_compute(
                "AllReduce", mybir.AluOpType.add,
                replica_groups=[list(range(NCORES))],
                ins=[w_in.opt()], outs=[w_out.opt()],
            )

            # small tensors via the scalar queue (not the bulk sync queue)
            identh = mp.tile([C, C], f16, name="identh_sb")
            nc.scalar.dma_start(identh[:], idh_d[:])
            ident = mp.tile([C, C], f32, name="ident_sb")
            nc.scalar.dma_start(ident[:], id_d[:])
            gcol = mp.tile([C, 1], f32, name="gcol")
            nc.scalar.dma_start(gcol[:], g_d[:])

            # ---- phase 1: per-batch E1/M accumulation + xb16 rebuild ----
            xb16 = [[] for _ in range(B)]
            e_cat = mp.tile([C, B * C], f32, name="e_cat")
            junk = mp.tile([C, 1], f16, name="junk")
            hi_ts, lo_ts = {}, {}
            acc = {}

            def emit_chunk(ch):
                b, cb = divmod(ch, NCHUNK_B)
                if cb == 0:
                    acc[b] = (eps.tile([C, C], f32, name=f"e1_ps{b}", tag="e"),
                              eps.tile([C, C], f32, name=f"m_ps{b}", tag="e"))
                e1_ps, m_ps = acc[b]
                hi_t, lo_t = hi_ts[ch], lo_ts[ch]
                xb = xbp.tile([C, CHUNK], f16, name=f"xb_{ch}", tag="xb")
                for k in range(TPC):
                    kk = slice(k * C, (k + 1) * C)
                    st = cb == 0 and k == 0
                    sp = cb == NCHUNK_B - 1 and k == TPC - 1
                    nc.tensor.matmul(e1_ps[:], hi_t[:, kk], hi_t[:, kk],
                                     start=st, stop=sp)
                    nc.tensor.matmul(m_ps[:], hi_t[:, kk], lo_t[:, kk],
                                     start=st, stop=sp)
                    tp = tps.tile([C, C], f16, name=f"tp_{ch}_{k}", tag="tp")
                    nc.tensor.transpose(tp[:], hi_t[:, kk], identh[:])
                    if k % 2 == 0:
                        nc.scalar.copy(xb[:, kk], tp[:])
                    else:
                        nc.vector.tensor_copy(xb[:, kk], tp[:])
                xb16[b].append(xb)
                if cb == NCHUNK_B - 1:
                    # E_b = E1 + M + M^T into e_cat[:, b*C:(b+1)*C].
                    # Order frees the accumulation banks fast: add1 releases
                    # e1_ps before the M-transpose lands.
                    ec = e_cat[:, b * C:(b + 1) * C]
                    m_sb = mp.tile([C, C], f32, name=f"m_sb{b}")
                    nc.vector.tensor_copy(m_sb[:], m_ps[:])
                    nc.vector.tensor_add(ec, e1_ps[:], m_sb[:])
                    mt_ps = tp2p.tile([C, C], f32, name=f"mt_ps{b}", tag="tq")
                    nc.tensor.transpose(mt_ps[:], m_sb[:], ident[:])
                    nc.vector.tensor_add(ec, ec, mt_ps[:])

            for ch in range(NCHUNK):
                hi_t = hip.tile([C, CHUNK], f16, name=f"hi_{ch}", tag="hi")
                nc.sync.dma_start(hi_t[:], hi_d[ch, :, :])
                hi_ts[ch] = hi_t
                lo_t = lop.tile([C, CHUNK], f16, name=f"lo_{ch}", tag="lo")
                nc.scalar.dma_start(lo_t[:], lo_d[ch, :, :])
                lo_ts[ch] = lo_t
                if ch == DELAY:
                    # PE start gate: consume chunk DELAY's tail so the PE
                    # sits idle until enough input is buffered, then runs
                    # phase 1 gap-free (p-state ramps to 2.4GHz and stays).
                    dtp = tps.tile([C, C], f16, name="dummy_tp", tag="tp")
                    nc.tensor.transpose(dtp[:], hi_t[:, CHUNK - C:], identh[:])
                    nc.vector.tensor_copy(junk[:], dtp[:, 0:1])
                    for ch2 in range(DELAY + 1):
                        emit_chunk(ch2)
                elif ch > DELAY:
                    emit_chunk(ch)

            # ---- single AllReduce for both batches ----
            ar_in = dramp.tile([C, B * C], f32, name="ar_in")
            ar_out = dramp.tile([C, B * C], f32, name="ar_out",
                                addr_space="Shared")
            nc.gpsimd.dma_start(ar_in[:], e_cat[:])
            nc.gpsimd.collective_compute(
                "AllReduce", mybir.AluOpType.add,
                replica_groups=[list(range(NCORES))],
                ins=[ar_in.opt()], outs=[ar_out.opt()],
            )
            e_red = mp.tile([C, B * C], f32, name="e_red")
            nc.gpsimd.dma_start(e_red[:], ar_out[:])

            # ---- phase 2: softmax + apply, per batch ----
            def emit_softmax(b):
                E_b = e_red[:, b * C:(b + 1) * C]
                mcol = mp.tile([C, 1], f32, name=f"mcol{b}")
                nc.vector.tensor_reduce(mcol[:], E_b, axis=mybir.AxisListType.X,
                                        op=mybir.AluOpType.min)
                P_b = mp.tile([C, C], f16, name=f"P{b}")
                zcol = mp.tile([C, 1], f32, name=f"zcol{b}")
                # P = exp(min_row - E), zcol = rowsum(P); exponents <= 0.
                # P's diagonal is exp(min - ~+147000) == 0 exactly.
                nc.scalar.activation(P_b[:], E_b,
                                     mybir.ActivationFunctionType.Exp,
                                     bias=mcol[:], scale=-1.0,
                                     accum_out=zcol[:])
                tpP = tps.tile([C, C], f16, name=f"tpP{b}", tag="tp")
                nc.tensor.transpose(tpP[:], P_b[:], identh[:])
                attnT = mp.tile([C, C], f16, name=f"attnT{b}")
                nc.vector.tensor_copy(attnT[:], tpP[:])
                # off the critical path: scol = gamma / Z
                rz = mp.tile([C, 1], f32, name=f"rz{b}")
                nc.vector.reciprocal(rz[:], zcol[:])
                scol = mp.tile([C, 1], f32, name=f"scol{b}")
                nc.vector.tensor_tensor(scol[:], rz[:], gcol[:],
                                        op=mybir.AluOpType.mult)
                return attnT, scol

            sm = [emit_softmax(b) for b in range(B)]
            for b in range(B):
                attnT, scol = sm[b]
                for cb in range(NCHUNK_B):
                    ost = ostp.tile([C, CHUNK], f16, name=f"ost_{b}_{cb}",
                                    tag="ost")
                    for j in range(CHUNK // OTILE):
                        op = ops.tile([C, OTILE], f32, name=f"op_{b}_{cb}_{j}",
                                      tag="op")
                        nc.tensor.matmul(
                            op[:], attnT[:],
                            xb16[b][cb][:, j * OTILE:(j + 1) * OTILE],
                            start=True, stop=True)
                        # ost = (P@q) * (gamma/Z) + q   (residual + row scale)
                        nc.vector.scalar_tensor_tensor(
                            ost[:, j * OTILE:(j + 1) * OTILE], op[:], scol[:],
                            xb16[b][cb][:, j * OTILE:(j + 1) * OTILE],
                            op0=mybir.AluOpType.mult, op1=mybir.AluOpType.add)
                    dst = o_d[b, :, cb * CHUNK:(cb + 1) * CHUNK]
                    if cb % 2 == 0:
                        nc.sync.dma_start(dst, ost[:])
                    else:
                        nc.scalar.dma_start(dst, ost[:])

    _log("tile context done; bacc compile start")
    nc.compile()
    _log("bacc compile done")
    return nc


def kernel(x, gamma, _trace=False, _tmpdir=None):
    from concourse import bass_utils

    x = np.ascontiguousarray(np.asarray(x), dtype=np.float32)
    gamma = np.asarray(gamma, dtype=np.float32)
    q = x.reshape(B, C, N)
    gcol = np.full((C, 1), gamma[0], dtype=np.float32)
    ident = np.eye(C, dtype=np.float32)
    identh = np.eye(C, dtype=np.float16)

    def blocked(a):
        # [B, C, NLOC] -> [B*NCHUNK_B, 128 n-partitions, TPC*C] SBUF image:
        # out[b*9+cb, p, k*C + c] = a[b, c, cb*CHUNK + k*C + p]
        t = a.reshape(B, C, NCHUNK_B, TPC, C).transpose(0, 2, 4, 3, 1)
        return np.ascontiguousarray(t.reshape(NCHUNK, C, CHUNK))

    in_maps = []
    for r in range(NCORES):
        qs = q[:, :, r * NLOC:(r + 1) * NLOC]
        hi = qs.astype(np.float16)
        lo = (qs - hi.astype(np.float32)).astype(np.float16)
        in_maps.append({
            "hi": blocked(hi),
            "lo": blocked(lo),
            "gamma_col": gcol,
            "ident": ident,
            "identh": identh,
        })

    nc = _get_nc()
    _log("launching run_bass_kernel_spmd")
    res = bass_utils.run_bass_kernel_spmd(
        nc, in_maps, core_ids=list(range(NCORES)), trace=_trace,
        tmpdir=_tmpdir)
    outs = [res.results[r]["out"] for r in range(NCORES)]
    full = np.concatenate(outs, axis=2).astype(np.float32)
    full = full.reshape(B, C, D, H, W)
    if _trace:
        return full.astype(np.float32, copy=False), res
    return full.astype(np.float32, copy=False)


def _get_nc():
    if "nc" not in _compiled:
        _compiled["nc"] = _build()
    return _compiled["nc"]


# revision 11
# speedup vs baseline: 1.2305x; 1.2305x over previous
"""Channel-attention module kernel for 8 Trainium2 NeuronCores.

reference semantics (B=2, C=128, N=D*H*W=147456):
    q = x.reshape(B, C, N)
    energy = q @ q^T                  # [B, C, C]
    attn = softmax(rowmax(energy) - energy, axis=-1)
          = softmax(-energy, axis=-1)             (rowmax shift is a no-op)
    out = attn @ q
    return x + gamma * out

Sharding: sequence-parallel over N. Core r owns columns
[r*N/8, (r+1)*N/8) of q for both batches. Each core computes a partial
energy (contraction over its local n), ONE AllReduce sums the two tiny
[C, C] energies (packed as [C, 2C]) across the 8 cores, each core then
computes the softmax redundantly and applies the attention to its
local columns.

Design notes (v3):
  - Host pre-splits q into f16 hi/lo (q = hi + lo + O(2^-22)) shipped
    PRE-TRANSPOSED in an SBUF-image blocked layout. Phase 1 is pure
    f16 PE work (1 cyc/row): E1 += hi_t^T hi_t, M += hi_t^T lo_t per
    128-tile; cross term completed once per batch as E = E1 + M + M^T.
    End-to-end rel err 3.3e-4 (numpy-validated; gate 2e-2).
  - [c, n] f16 q copy for phase 2 rebuilt on-chip via f16 PE
    transposes; PSUM->SBUF copies split ScalarE/DVE.
  - P-state: TRN2 PE runs 1.2GHz until ~3us of CONTINUOUS busy, then
    2.4GHz. A DMA-paced phase 1 keeps stalling PE and pins it at
    1.2GHz (measured). So PE start is delayed (dummy transpose gated
    on chunk DELAY's arrival) until enough input is buffered that PE
    then runs gap-free at 2.4GHz to the end of phase 1.
  - hi loads on the sync HWDGE queue, lo loads on the scalar queue,
    output stores alternate sync/scalar: two queues split the HBM
    traffic. Small tensors (identities, gamma) load via the scalar
    queue so they don't head-of-line block the bulk stream.
  - ONE AllReduce [C, 2C] fp32; warm-up collective fired dep-free at
    t=0 (the first collective pays a ~60-85us ncfw cold-start
    pipeline; hw-measured).
  - softmax critical path is just min -> exp(accum Z) -> transpose:
    the gamma/Z row scale and the +q residual are fused into the
    phase-2 PSUM->SBUF copies as (op * scol) + q_f16 on DVE.
"""

import sys

sys.path.insert(0, "/opt/trn_rl_repo")

import numpy as np

B, C = 2, 128
D, H, W = 16, 96, 96
N = D * H * W  # 147456
NCORES = 8
NLOC = N // NCORES  # 18432
CHUNK = 2048
NCHUNK_B = NLOC // CHUNK  # 9 chunks per batch
NCHUNK = B * NCHUNK_B  # 18
TPC = CHUNK // C  # 16 n-tiles of 128 per chunk
OTILE = 512
GRP = 8 * C  # 1024: transpose-group / phase-2 copy granularity
DELAY = 3  # chunks buffered before PE starts (p-state ramp guard)

_compiled = {}


def _log(msg):
    import time as _t
    print(f"[kernel {_t.strftime('%H:%M:%S')}] {msg}", flush=True)


def _build():
    import concourse.bacc as bacc
    import concourse.tile as tile
    import concourse.mybir as mybir

    _log("build start")

    f32 = mybir.dt.float32
    f16 = mybir.dt.float16
    nc = bacc.Bacc("TRN2", target_bir_lowering=False, debug=False,
                   num_devices=NCORES)

    hi_d = nc.dram_tensor("hi", [NCHUNK, C, CHUNK], f16,
                          kind="ExternalInput").ap()
    lo_d = nc.dram_tensor("lo", [NCHUNK, C, CHUNK], f16,
                          kind="ExternalInput").ap()
    g_d = nc.dram_tensor("gamma_col", [C, 1], f32, kind="ExternalInput").ap()
    id_d = nc.dram_tensor("ident", [C, C], f32, kind="ExternalInput").ap()
    idh_d = nc.dram_tensor("identh", [C, C], f16, kind="ExternalInput").ap()
    o_d = nc.dram_tensor("out", [B, C, NLOC], f16, kind="ExternalOutput").ap()

    with tile.TileContext(nc) as tc:
        with (
            tc.tile_pool(name="hip", bufs=8) as hip,
            tc.tile_pool(name="lop", bufs=8) as lop,
            tc.tile_pool(name="xb16", bufs=NCHUNK) as xbp,
            tc.tile_pool(name="tps", bufs=2, space="PSUM") as tps,
            tc.tile_pool(name="tp2", bufs=1, space="PSUM") as tp2p,
            tc.tile_pool(name="eps", bufs=2, space="PSUM") as eps,
            tc.tile_pool(name="ops", bufs=3, space="PSUM") as ops,
            tc.tile_pool(name="misc", bufs=1) as mp,
            tc.tile_pool(name="ost", bufs=4) as ostp,
            tc.tile_pool(name="dram", bufs=1, space="DRAM") as dramp,
        ):
            # ---- warm-up collective, dependency-free, at t=0 ----
            wz = mp.tile([C, 1], f32, name="wz")
            nc.gpsimd.memzero(wz[:])
            w_in = dramp.tile([C, 1], f32, name="w_in")
            w_out = dramp.tile([C, 1], f32, name="w_out", addr_space="Shared")
            nc.gpsimd.dma_start(w_in[:], wz[:])
            nc.gpsimd.collective_compute(
                "AllReduce", mybir.AluOpType.add,
                replica_groups=[list(range(NCORES))],
                ins=[w_in.opt()], outs=[w_out.opt()],
            )

            # small tensors via the scalar queue (not the bulk sync queue)
            identh = mp.tile([C, C], f16, name="identh_sb")
            nc.scalar.dma_start(identh[:], idh_d[:])
            ident = mp.tile([C, C], f32, name="ident_sb")
            nc.scalar.dma_start(ident[:], id_d[:])
            gcol = mp.tile([C, 1], f32, name="gcol")
            nc.scalar.dma_start(gcol[:], g_d[:])

            # ---- phase 1: per-batch E1/M accumulation + xb16 rebuild ----
            xb16 = [[] for _ in range(B)]
            e_cat = mp.tile([C, B * C], f32, name="e_cat")
            junk = mp.tile([C, 1], f16, name="junk")
            hi_ts, lo_ts = {}, {}
            acc = {}

            def emit_chunk(ch):
                b, cb = divmod(ch, NCHUNK_B)
                if cb == 0:
                    acc[b] = (eps.tile([C, C], f32, name=f"e1_ps{b}", tag="e"),
                              eps.tile([C, C], f32, name=f"m_ps{b}", tag="e"))
                e1_ps, m_ps = acc[b]
                hi_t, lo_t = hi_ts[ch], lo_ts[ch]
                xb = xbp.tile([C, CHUNK], f16, name=f"xb_{ch}", tag="xb")
                for g in range(CHUNK // GRP):
                    # 8 transposes land in one [C, GRP] PSUM tile so the
                    # PSUM->SBUF copy is one instruction (per-op overheads
                    # on ScalarE/DVE dwarf 128-col copies).
                    tp = tps.tile([C, GRP], f16, name=f"tp_{ch}_{g}",
                                  tag="tp")
                    for kg in range(GRP // C):
                        k = g * (GRP // C) + kg
                        kk = slice(k * C, (k + 1) * C)
                        st = cb == 0 and k == 0
                        sp = cb == NCHUNK_B - 1 and k == TPC - 1
                        nc.tensor.matmul(e1_ps[:], hi_t[:, kk], hi_t[:, kk],
                                         start=st, stop=sp)
                        nc.tensor.matmul(m_ps[:], hi_t[:, kk], lo_t[:, kk],
                                         start=st, stop=sp)
                        nc.tensor.transpose(tp[:, kg * C:(kg + 1) * C],
                                            hi_t[:, kk], identh[:])
                    dst = xb[:, g * GRP:(g + 1) * GRP]
                    if g % 2 == 0:
                        nc.scalar.copy(dst, tp[:])
                    else:
                        nc.vector.tensor_copy(dst, tp[:])
                xb16[b].append(xb)
                if cb == NCHUNK_B - 1:
                    # E_b = E1 + M + M^T into e_cat[:, b*C:(b+1)*C].
                    # Order frees the accumulation banks fast: add1 releases
                    # e1_ps before the M-transpose lands.
                    ec = e_cat[:, b * C:(b + 1) * C]
                    m_sb = mp.tile([C, C], f32, name=f"m_sb{b}")
                    nc.vector.tensor_copy(m_sb[:], m_ps[:])
                    nc.vector.tensor_add(ec, e1_ps[:], m_sb[:])
                    mt_ps = tp2p.tile([C, C], f32, name=f"mt_ps{b}", tag="tq")
                    nc.tensor.transpose(mt_ps[:], m_sb[:], ident[:])
                    nc.vector.tensor_add(ec, ec, mt_ps[:])

            for ch in range(NCHUNK):
                hi_t = hip.tile([C, CHUNK], f16, name=f"hi_{ch}", tag="hi")
                nc.sync.dma_start(hi_t[:], hi_d[ch, :, :])
                hi_ts[ch] = hi_t
                lo_t = lop.tile([C, CHUNK], f16, name=f"lo_{ch}", tag="lo")
                nc.scalar.dma_start(lo_t[:], lo_d[ch, :, :])
                lo_ts[ch] = lo_t
                if ch == DELAY:
                    # PE start gate: consume chunk DELAY's tail so the PE
                    # sits idle until enough input is buffered, then runs
                    # phase 1 gap-free (p-state ramps to 2.4GHz and stays).
                    dtp = tp2p.tile([C, C], f16, name="dummy_tp", tag="tq")
                    nc.tensor.transpose(dtp[:], hi_t[:, CHUNK - C:], identh[:])
                    nc.vector.tensor_copy(junk[:], dtp[:, 0:1])
                if ch >= DELAY:
                    emit_chunk(ch - DELAY)
            for ch in range(NCHUNK - DELAY, NCHUNK):
                emit_chunk(ch)

            # ---- single AllGather for both batches + on-chip sum ----
            # AG floor is ~4.6us vs AllReduce's ~9.7 (8 cores, 1 chip); the
            # 8-way sum of the tiny [C, 2C] partials is ~2us of DVE.
            ag_in = dramp.tile([C, B * C], f32, name="ag_in")
            ag_out = dramp.tile([NCORES, C, B * C], f32, name="ag_out",
                                addr_space="Shared")
            nc.gpsimd.dma_start(ag_in[:], e_cat[:])
            nc.gpsimd.collective_compute(
                "AllGather", mybir.AluOpType.bypass,
                replica_groups=[list(range(NCORES))],
                ins=[ag_in.opt()], outs=[ag_out.opt()],
            )
            e_all = mp.tile([C, NCORES * B * C], f32, name="e_all")
            for r in range(NCORES):
                nc.sync.dma_start(
                    e_all[:, r * B * C:(r + 1) * B * C], ag_out[r, :, :])
            h = NCORES * B * C // 2  # 1024
            nc.vector.tensor_add(e_all[:, 0:h], e_all[:, 0:h],
                                 e_all[:, h:2 * h])
            nc.vector.tensor_add(e_all[:, 0:h // 2], e_all[:, 0:h // 2],
                                 e_all[:, h // 2:h])
            e_red = mp.tile([C, B * C], f32, name="e_red")
            nc.vector.tensor_add(e_red[:], e_all[:, 0:B * C],
                                 e_all[:, B * C:2 * B * C])

            # ---- phase 2: softmax + apply, per batch ----
            def emit_softmax(b):
                E_b = e_red[:, b * C:(b + 1) * C]
                mcol = mp.tile([C, 1], f32, name=f"mcol{b}")
                nc.vector.tensor_reduce(mcol[:], E_b, axis=mybir.AxisListType.X,
                                        op=mybir.AluOpType.min)
                P_b = mp.tile([C, C], f16, name=f"P{b}")
                zcol = mp.tile([C, 1], f32, name=f"zcol{b}")
                # P = exp(min_row - E), zcol = rowsum(P); exponents <= 0.
                # P's diagonal is exp(min - ~+147000) == 0 exactly.
                nc.scalar.activation(P_b[:], E_b,
                                     mybir.ActivationFunctionType.Exp,
                                     bias=mcol[:], scale=-1.0,
                                     accum_out=zcol[:])
                rz = mp.tile([C, 1], f32, name=f"rz{b}")
                nc.vector.reciprocal(rz[:], zcol[:])
                scol = mp.tile([C, 1], f32, name=f"scol{b}")
                nc.vector.tensor_tensor(scol[:], rz[:], gcol[:],
                                        op=mybir.AluOpType.mult)
                # attn_s = (gamma/Z) * P + I -> matmul computes x + gamma*attn@q
                nc.vector.tensor_scalar_mul(P_b[:], P_b[:], scol[:])
                nc.vector.tensor_add(P_b[:], P_b[:], identh[:])
                tpP = tp2p.tile([C, C], f16, name=f"tpP{b}", tag="tq")
                nc.tensor.transpose(tpP[:], P_b[:], identh[:])
                attnT = mp.tile([C, C], f16, name=f"attnT{b}")
                nc.vector.tensor_copy(attnT[:], tpP[:])
                return attnT

            sm = [emit_softmax(b) for b in range(B)]
            for b in range(B):
                attnT = sm[b]
                for cb in range(NCHUNK_B):
                    xb = xb16[b][cb]
                    ost = ostp.tile([C, CHUNK], f16, name=f"ost_{b}_{cb}",
                                    tag="ost")
                    for j in range(CHUNK // OTILE):
                        jj = slice(j * OTILE, (j + 1) * OTILE)
                        op = ops.tile([C, OTILE], f32, name=f"op_{b}_{cb}_{j}",
                                      tag="op")
                        nc.tensor.matmul(op[:], attnT[:], xb[:, jj],
                                         start=True, stop=True)
                        if j % 2 == 0:
                            nc.vector.tensor_copy(ost[:, jj], op[:])
                        else:
                            nc.scalar.copy(ost[:, jj], op[:])
                    dst = o_d[b, :, cb * CHUNK:(cb + 1) * CHUNK]
                    if cb % 2 == 0:
                        nc.sync.dma_start(dst, ost[:])
                    else:
                        nc.scalar.dma_start(dst, ost[:])

    _log("tile context done; bacc compile start")
    nc.compile()
    _log("bacc compile done")
    return nc


def kernel(x, gamma, _trace=False, _tmpdir=None):
    from concourse import bass_utils

    x = np.ascontiguousarray(np.asarray(x), dtype=np.float32)
    gamma = np.asarray(gamma, dtype=np.float32)
    q = x.reshape(B, C, N)
    gcol = np.full((C, 1), gamma[0], dtype=np.float32)
    ident = np.eye(C, dtype=np.float32)
    identh = np.eye(C, dtype=np.float16)

    def blocked(a):
        # [B, C, NLOC] -> [B*NCHUNK_B, 128 n-partitions, TPC*C] SBUF image:
        # out[b*9+cb, p, k*C + c] = a[b, c, cb*CHUNK + k*C + p]
        t = a.reshape(B, C, NCHUNK_B, TPC, C).transpose(0, 2, 4, 3, 1)
        return np.ascontiguousarray(t.reshape(NCHUNK, C, CHUNK))

    in_maps = []
    for r in range(NCORES):
        qs = q[:, :, r * NLOC:(r + 1) * NLOC]
        hi = qs.astype(np.float16)
        lo = (qs - hi.astype(np.float32)).astype(np.float16)
        in_maps.append({
            "hi": blocked(hi),
            "lo": blocked(lo),
            "gamma_col": gcol,
            "ident": ident,
            "identh": identh,
        })

    nc = _get_nc()
    _log("launching run_bass_kernel_spmd")
    res = bass_utils.run_bass_kernel_spmd(
        nc, in_maps, core_ids=list(range(NCORES)), trace=_trace,
        tmpdir=_tmpdir)
    outs = [res.results[r]["out"] for r in range(NCORES)]
    full = np.concatenate(outs, axis=2).astype(np.float32)
    full = full.reshape(B, C, D, H, W)
    if _trace:
        return full.astype(np.float32, copy=False), res
    return full.astype(np.float32, copy=False)


def _get_nc():
    if "nc" not in _compiled:
        _compiled["nc"] = _build()
    return _compiled["nc"]
